# revision 16
# baseline (speedup 1.0000x reference)
"""Trainium2 Bass kernel: LayerNorm -> attention-score -> softmax(seq) -> weighted pooling.

Reference computation (per sample b):
    normed = LayerNorm(x[b])                       # over H
    scores = normed @ w                            # [S]
    weights = softmax(clip(scores - max, -10, 10)) # over S
    out[b]  = weights @ normed                     # [H]

Factorization used here (exact, validated vs reference to ~1e-6):
    score_s = (s3_s - C1*mu_s) * rstd_s   (+ C2, constant -> cancels in softmax)
      where s1 = sum_h x, s2 = sum_h x^2, s3 = sum_h x*(gamma*w),
            mu = s1/H, var = s2/H - mu^2, rstd = 1/sqrt(var+eps), C1 = sum gamma*w
    alpha_s = exp(max(score_s - M, -10)) * rstd_s     (M = max_s score)
    out_h   = gamma_h * (sum_s alpha_s * x_sh - sum_s alpha_s*mu_s) / Z + beta_h
      where Z = sum_s exp(max(score_s - M, -10))

Single pass over the 512MB input: each sample's 16MB is streamed into SBUF in
1MB slots, per-token stats are computed while resident, softmax is done exactly
on-chip, then TensorE matmuls (alpha-weighted token sums, float32r for 1
cycle/row) consume the same resident tiles. Per-token reductions are load-
balanced: the first NV_TILES token-tiles of each sample use VectorE bn_stats
for (mean, var), the rest use ScalarE activation-accumulate (sum, sum-of-
squares). Data-parallel over batch: 4 samples per NeuronCore x 8 cores.
"""

import os
import sys
from contextlib import ExitStack

import numpy as np

for _p in ("/opt/trn_rl_repo", "/root/.axon_site/_ro/trn_rl_repo"):
    if os.path.isdir(_p) and _p not in sys.path:
        sys.path.insert(0, _p)

import concourse.bass as bass
import concourse.tile as tile
from concourse import bacc, mybir
from concourse.bass_utils import run_bass_kernel_spmd

F32 = mybir.dt.float32
F32R = mybir.dt.float32r
AF = mybir.ActivationFunctionType
ALU = mybir.AluOpType
AX = mybir.AxisListType

B, S, H = 32, 4096, 1024
NCORES = 8
BL = B // NCORES            # samples per core
P = 128                     # partitions (tokens per token-tile)
HHALF = H // 2
EPS = 1e-5

TPT = S // P                # 32 token-tiles per sample
SLOT_TT = 2                 # token-tiles per DMA slot (1MB per DMA)
NSLOTS = TPT // SLOT_TT     # 16 slots per sample
RING = 17                   # x ring buffers (16 = one full sample + 1 prefetch)
NV_TILES = 20               # tiles per sample doing (mean,var) on VectorE bn_stats;
                            # the rest use ScalarE act-accumulate (s1, s2)


def _build(c1: float):
    nc = bacc.Bacc(None)

    x_ext = nc.declare_dram_parameter("x", [BL, S, H], F32R, isOutput=False)
    gwb_ext = nc.declare_dram_parameter("gwb", [P, H], F32, isOutput=False)
    gb_ext = nc.declare_dram_parameter("gb", [1, 2 * H], F32, isOutput=False)
    id_ext = nc.declare_dram_parameter("ident", [P, P], F32, isOutput=False)
    out_ext = nc.declare_dram_parameter("out", [BL, H], F32, isOutput=True)

    with ExitStack() as ctx:
        tc = ctx.enter_context(tile.TileContext(nc))
        xpool = ctx.enter_context(tc.tile_pool(name="xring", bufs=RING))
        consts = ctx.enter_context(tc.tile_pool(name="consts", bufs=1))
        scr = ctx.enter_context(tc.tile_pool(name="scr", bufs=2))
        small = ctx.enter_context(tc.tile_pool(name="small", bufs=2))
        epi = ctx.enter_context(tc.tile_pool(name="epi", bufs=1))
        stats = ctx.enter_context(tc.tile_pool(name="stats", bufs=1))
        pscr = ctx.enter_context(
            tc.tile_pool(name="pscr", bufs=3, space=bass.MemorySpace.PSUM)
        )
        pacc_pool = ctx.enter_context(
            tc.tile_pool(name="pacc", bufs=2, space=bass.MemorySpace.PSUM)
        )

        gwb = consts.tile([P, H], F32)
        nc.sync.dma_start(gwb[:], gwb_ext[:])
        ident = consts.tile([P, P], F32)
        nc.sync.dma_start(ident[:], id_ext[:])
        gb = consts.tile([1, 2 * H], F32)
        nc.sync.dma_start(gb[:], gb_ext[:])
        ones_row = consts.tile([1, P], F32)
        nc.vector.memset(ones_row[:], 1.0)
        epsb = consts.tile([P, 1], F32)
        nc.vector.memset(epsb[:], EPS)

        # persistent per-token stat buffers (columns: b*TPT + tile)
        scores = stats.tile([P, BL * TPT], F32, tag="scores")
        mv = stats.tile([P, BL * TPT, 2], F32, tag="mv")      # (mean, var)
        s3b = stats.tile([P, BL * TPT], F32, tag="s3b")       # sum x*gw
        rstd = stats.tile([P, BL * TPT], F32, tag="rstd")
        abuf = stats.tile([P, BL * TPT], F32R, tag="abuf")     # alpha weights

        for b in range(BL):
            # ---------------- stage A: stream + per-token reductions ----------------
            slot_aps = []
            for sl in range(NSLOTS):
                xt = xpool.tile([P, SLOT_TT * H], F32R, tag="xt")
                slot_aps.append(xt)
                s0 = sl * SLOT_TT * P
                src = x_ext[b, s0 : s0 + SLOT_TT * P, :].rearrange(
                    "(tt p) h -> p tt h", p=P
                )
                dst = xt[:].rearrange("p (tt h) -> p tt h", h=H)
                nc.sync.dma_start(out=dst, in_=src)

                for t in range(SLOT_TT):
                    col = b * TPT + sl * SLOT_TT + t
                    tile_in_sample = sl * SLOT_TT + t
                    xv = xt[:, t * H : (t + 1) * H].bitcast(F32)
                    # s3 = sum_h x*gw : VectorE multiply + ScalarE accum-reduce
                    yv = scr.tile([P, H], F32, tag="yv")
                    nc.vector.tensor_tensor(yv[:], xv, gwb[:], ALU.mult)
                    ys = scr.tile([P, H], F32, tag="ys")
                    nc.scalar.activation(
                        ys[:],
                        yv[:],
                        AF.Identity,
                        accum_out=s3b[:, col : col + 1],
                    )
                    if tile_in_sample < NV_TILES:
                        # (mean, var) on VectorE
                        st6 = scr.tile([P, 2, 6], F32, tag="st6")
                        nc.vector.bn_stats(st6[:, 0, :], xv[:, :HHALF])
                        nc.vector.bn_stats(st6[:, 1, :], xv[:, HHALF:])
                        nc.vector.bn_aggr(mv[:, col, :], st6[:])
                    else:
                        # raw s1, s2 on ScalarE (converted to mean/var below)
                        ys1 = scr.tile([P, H], F32, tag="ys1")
                        nc.scalar.activation(
                            ys1[:], xv, AF.Identity, accum_out=mv[:, col, 0:1]
                        )
                        ys2 = scr.tile([P, H], F32, tag="ys2")
                        nc.scalar.activation(
                            ys2[:], xv, AF.Square, accum_out=mv[:, col, 1:2]
                        )

            bcols = slice(b * TPT, (b + 1) * TPT)
            # convert raw (s1, s2) -> (mean, var) for the ScalarE-typed tiles
            if NV_TILES < TPT:
                ns = TPT - NV_TILES
                sc0 = b * TPT + NV_TILES
                mu_s = mv[:, sc0 : sc0 + ns, 0]
                v_s = mv[:, sc0 : sc0 + ns, 1]
                nc.vector.tensor_scalar_mul(mu_s, mu_s, 1.0 / H)
                musq = small.tile([P, ns], F32, tag="musq")
                nc.scalar.activation(musq[:], mu_s, AF.Square)
                nc.vector.tensor_scalar_mul(v_s, v_s, 1.0 / H)
                nc.vector.tensor_tensor(v_s, v_s, musq[:], ALU.subtract)

            # batched score combine: score = (s3 - C1*mu) * rstd
            sd32 = small.tile([P, TPT], F32, tag="sd32")
            nc.scalar.activation(sd32[:], mv[:, bcols, 1], AF.Sqrt, bias=epsb[:])
            nc.vector.reciprocal(rstd[:, bcols], sd32[:])
            tmp32 = small.tile([P, TPT], F32, tag="tmp32")
            nc.vector.tensor_scalar_mul(tmp32[:], mv[:, bcols, 0], c1)
            u32 = small.tile([P, TPT], F32, tag="u32")
            nc.vector.tensor_tensor(u32[:], s3b[:, bcols], tmp32[:], ALU.subtract)
            nc.vector.tensor_tensor(scores[:, bcols], u32[:], rstd[:, bcols], ALU.mult)

            # ---------------- stage B: exact softmax over sample b ----------------
            m1 = small.tile([P, 1], F32, tag="m1")
            nc.vector.tensor_reduce(m1[:], scores[:, bcols], AX.X, ALU.max)
            tp = pscr.tile([1, P], F32, tag="pss")
            nc.tensor.transpose(tp[:], m1[:], ident[:])
            mx = small.tile([1, 1], F32, tag="mx")
            nc.vector.tensor_reduce(mx[:], tp[:], AX.X, ALU.max)
            neg_m = small.tile([1, 1], F32, tag="neg_m")
            nc.scalar.mul(neg_m[:], mx[:], -1.0)
            mb = pscr.tile([P, 1], F32, tag="pss")
            nc.tensor.matmul(mb[:], ones_row[:], neg_m[:])
            neg_mb = small.tile([P, 1], F32, tag="neg_mb")
            nc.vector.tensor_copy(neg_mb[:], mb[:])
            sh4 = small.tile([P, TPT], F32, tag="sh4")
            nc.scalar.activation(sh4[:], scores[:, bcols], AF.Identity, bias=neg_mb[:])
            nc.vector.tensor_scalar_max(sh4[:], sh4[:], -10.0)
            e4 = small.tile([P, TPT], F32, tag="e4")
            nc.scalar.activation(e4[:], sh4[:], AF.Exp)
            nc.vector.tensor_tensor(abuf[:, bcols], e4[:], rstd[:, bcols], ALU.mult)
            # qz col0 = partial Dr = sum alpha*mu, col1 = partial Z = sum e
            qz = small.tile([P, 2], F32, tag="qz")
            ttq = small.tile([P, TPT], F32, tag="ttq")
            nc.vector.tensor_tensor(ttq[:], abuf[:, bcols].bitcast(F32), mv[:, bcols, 0], ALU.mult)
            nc.vector.tensor_reduce(qz[:, 0:1], ttq[:], AX.X, ALU.add)
            nc.vector.tensor_reduce(qz[:, 1:2], e4[:], AX.X, ALU.add)
            tq = pscr.tile([2, P], F32, tag="pss")
            nc.tensor.transpose(tq[:], qz[:], ident[:])
            dz = small.tile([2, 1], F32, tag="dz")
            nc.vector.tensor_reduce(dz[:], tq[:], AX.X, ALU.add)
            # bring (Dr, Z) onto partition 0 as [1,2] (partition starts must be 0/32/64/96)
            dzt_p = pscr.tile([1, 2], F32, tag="pss")
            nc.tensor.transpose(dzt_p[:], dz[:], ident[0:2, 0:2])
            dzt = small.tile([1, 2], F32, tag="dzt")
            nc.vector.tensor_copy(dzt[:], dzt_p[:])
            rz = small.tile([1, 1], F32, tag="rz")
            nc.vector.reciprocal(rz[:], dzt[0:1, 1:2])
            ndz = small.tile([1, 1], F32, tag="ndz")
            nc.vector.tensor_tensor(ndz[:], dzt[0:1, 0:1], rz[:], ALU.mult)
            ndz2 = small.tile([1, 1], F32, tag="ndz2")
            nc.scalar.mul(ndz2[:], ndz[:], -1.0)

            # ---------------- stage C: alpha-weighted pooling (float32r) ----------------
            pacc = pacc_pool.tile([1, H], F32, tag="pacc")
            for hh in range(2):
                h0 = hh * HHALF
                for sl in range(NSLOTS):
                    xt = slot_aps[sl]
                    for t in range(SLOT_TT):
                        col = b * TPT + sl * SLOT_TT + t
                        first = sl == 0 and t == 0
                        last = sl == NSLOTS - 1 and t == SLOT_TT - 1
                        nc.tensor.matmul(
                            pacc[:, h0 : h0 + HHALF],
                            abuf[:, col : col + 1],
                            xt[:, t * H + h0 : t * H + h0 + HHALF],
                            start=first,
                            stop=last,
                        )

            # ---------------- epilogue: out = gamma*(P - Dr)/Z + beta ----------------
            t1 = epi.tile([1, H], F32, tag="t1")
            nc.scalar.activation(t1[:], pacc[:], AF.Identity, scale=rz[:], bias=ndz2[:])
            t2 = epi.tile([1, H], F32, tag="t2")
            nc.vector.tensor_tensor(t2[:], t1[:], gb[0:1, 0:H], ALU.mult)
            t3 = epi.tile([1, H], F32, tag="t3")
            nc.vector.tensor_tensor(t3[:], t2[:], gb[0:1, H:], ALU.add)
            nc.sync.dma_start(out_ext[b : b + 1, :], t3[:])

    nc.compile()
    return nc


_CACHE: dict = {}
LAST = None  # last BassKernelResults (exec_time_ns etc), for test harness use


def kernel(lstm_output, ln_gamma, ln_beta, attn_w, _trace=False, _trace_kwargs=None):
    global LAST
    x = np.ascontiguousarray(np.asarray(lstm_output, dtype=np.float32))
    gamma = np.asarray(ln_gamma, dtype=np.float32)
    beta = np.asarray(ln_beta, dtype=np.float32)
    w = np.asarray(attn_w, dtype=np.float32)
    assert x.shape == (B, S, H)

    gw = gamma * w
    c1 = float(gw.sum())
    key = ("nc", round(c1, 10))
    if key not in _CACHE:
        _CACHE.clear()
        _CACHE[key] = _build(c1)
    nc = _CACHE[key]

    gwb = np.ascontiguousarray(np.broadcast_to(gw[None, :], (P, H)))
    gb = np.concatenate([gamma, beta])[None, :].copy()
    ident = np.eye(P, dtype=np.float32)

    shards = x.reshape(NCORES, BL, S, H)
    in_maps = [
        {"x": shards[i], "gwb": gwb, "gb": gb, "ident": ident} for i in range(NCORES)
    ]
    kwargs = {}
    if _trace:
        kwargs["trace"] = True
        if _trace_kwargs:
            kwargs.update(_trace_kwargs)
    LAST = run_bass_kernel_spmd(nc, in_maps, core_ids=list(range(NCORES)), **kwargs)
    out = np.concatenate([LAST.results[i]["out"] for i in range(NCORES)], axis=0)
    return out.astype(np.float32)
